# revision 26
# baseline (speedup 1.0000x reference)
"""Trainium2 Bass kernel for nn_SubspaceLinopFactory (subspace NUDFT forward).

Math (reference):
  s[a,c,h,w] = x[a,h,w] * mps[c,h,w]
  E[r,k,(h,w)] = exp(-i*(ty[k]*gy[h] + tx[k]*gx[w]))   (separable, per r)
  y[a,c,k] = sum_hw E * s ;  z[t,c,k] = sum_a phi[a,t]*y*dcf[k]
  out[t,c,k] = z from core r = subsamp_idx[t]

Sharding: trajectory r -> core r (R == 8 == n_cores).

Per-core pipeline (per 512-wide k-chunk):
  - one fused trig table [128,2,KC]: rows 0-63 y-phases, 64-127 x-phases.
    ScalarE Copy(scale=g/2pi per-partition, bias=16.5/16.75 for sin/cos
    halves), VectorE i32-cast, GpSimd mixed-dtype subtract -> frac,
    ScalarE Sin(scale=-2pi) -> fp16 sin/cos in one shot.
  - xtd = x-trig * dcf (VectorE), cos half DMA'd to partitions 0-63.
  - stage 1: row-tiled concurrent matmuls contract w=64: P (rows 0-63)
    and Q (rows 64-127) into one 2-bank PSUM tile; single ScalarE cast
    to fp16.
  - pair-products (broadcast AP): T1 = P x {sy,cy} = {D,A},
    T2 = Q x {sy,cy} = {B,C}   (VectorE, one j-slice on GpSimd).
  - h-reduction matmuls with phi pre-folded into +/-W [128,128] weights
    accumulate z[(t,c),k] re/im directly in PSUM; ScalarE cast to fp16,
    DMA out.
"""
import numpy as np

A, T, C, R, D, K, H, W = 3, 32, 4, 8, 2, 1024, 64, 64
N_CORES = 8
ACH = A * C * H          # 768
MT = ACH // 128          # 6 m-tiles
KC = 512
NKC = K // KC            # 2

_CACHE = {}


def _build_nc():
    import concourse.bacc as bacc
    import concourse.tile as tile
    import concourse.mybir as mybir

    AF = mybir.ActivationFunctionType
    OP = mybir.AluOpType
    F32 = mybir.dt.float32
    F16 = mybir.dt.float16
    I32 = mybir.dt.int32
    TWO_PI = float(2 * np.pi)

    nc = bacc.Bacc(None, target_bir_lowering=False)

    d_phase = nc.dram_tensor("phase", [2, K], F32, kind="ExternalInput")
    d_sc = nc.dram_tensor("sc", [128, 2], F32, kind="ExternalInput")
    d_sxm = nc.dram_tensor("sxm", [64, 2 * ACH], F16, kind="ExternalInput")
    d_dcf = nc.dram_tensor("dcf", [2, K], F16, kind="ExternalInput")
    d_wp = nc.dram_tensor("wp", [128, MT * 128], F16, kind="ExternalInput")
    d_z = nc.dram_tensor("z", [128, 2, K], F16, kind="ExternalOutput")

    with tile.TileContext(nc) as tc:
        with (
            tc.tile_pool(name="cst", bufs=1) as cst,
            tc.tile_pool(name="trig", bufs=2) as trig,
            tc.tile_pool(name="tbl", bufs=2) as tblp,
            tc.tile_pool(name="work", bufs=3) as work,
            tc.tile_pool(name="zo", bufs=2) as zop,
            tc.tile_pool(name="psPQ", bufs=3, space="PSUM") as psPQ,
            tc.tile_pool(name="psZ", bufs=1, space="PSUM") as psZ,
        ):
            sc = cst.tile([128, 2], F32)
            sxm = cst.tile([64, 2 * ACH], F16)
            wp = cst.tile([128, MT, 128], F16)
            wm = cst.tile([128, MT, 128], F16)
            sT = cst.tile([64, ACH], F16)

            # critical-path DMAs first on sync; bulk on gpsimd queue
            tin0 = trig.tile([128, KC], F32, tag="tin")
            nc.sync.dma_start(tin0[0:64, :],
                              d_phase[1:2, 0:KC].broadcast_to([64, KC]))
            nc.sync.dma_start(tin0[64:128, :],
                              d_phase[0:1, 0:KC].broadcast_to([64, KC]))
            dcfb0 = trig.tile([128, 2, KC], F16, tag="dcfb")
            nc.sync.dma_start(
                dcfb0[0:64, :, :],
                d_dcf[:, 0:KC].rearrange("(o a) k -> o a k", o=1).broadcast_to([64, 2, KC]))
            nc.gpsimd.dma_start(sc[:], d_sc[:])
            nc.gpsimd.dma_start(sxm[:], d_sxm[:])
            nc.gpsimd.dma_start(wp[:], d_wp[:])

            def trig_head(tin, feng):
                m = trig.tile([128, 2, KC], F32, tag="m")
                nc.vector.tensor_scalar(m[:, 0, :], tin[:], sc[:, 0:1], 16.5,
                                        OP.mult, OP.add)
                nc.vector.tensor_scalar(m[:, 1, :], tin[:], sc[:, 0:1], 16.75,
                                        OP.mult, OP.add)
                mi = trig.tile([128, 2, KC], I32, tag="mi")
                nc.vector.tensor_copy(mi[:], m[:])
                fr = trig.tile([128, 2, KC], F32, tag="fr")
                feng.tensor_tensor(fr[:], m[:], mi[:], OP.subtract)
                return fr

            def sin_op(fr):
                tbl = tblp.tile([128, 2, KC], F16, tag="tbl")
                nc.scalar.activation(tbl[:], fr[:], AF.Sin, scale=-TWO_PI)
                return tbl

            def xy_tables(tbl, dcfb, xeng, deng):
                xtd = tblp.tile([64, 2, KC], F16, tag="xtd")
                xeng.tensor_tensor(xtd[:], tbl[0:64, :, :],
                                   dcfb[0:64, :, :], OP.mult)
                ytab = tblp.tile([128, 2, KC], F16, tag="ytab")
                deng.dma_start(ytab[0:64, :, :], tbl[64:128, :, :])
                deng.dma_start(ytab[64:128, :, :], tbl[64:128, :, :])
                return xtd, ytab

            def st1(j, xtd):
                js = slice(j * 128, (j + 1) * 128)
                pq = psPQ.tile([128, 2, KC], F32, tag="pq")
                nc.tensor.matmul(pq[:, 0, :], sT[:, js],
                                 xtd[:, 1, :], start=True, stop=True)
                nc.tensor.matmul(pq[:, 1, :], sT[:, js],
                                 xtd[:, 0, :], start=True, stop=True)
                return pq

            def cast_pq(pq):
                pq16 = work.tile([128, 2, KC], F16, tag="pq16")
                nc.scalar.copy(pq16[:, 0, :], pq[:, 0, :])
                nc.scalar.copy(pq16[:, 1, :], pq[:, 1, :])
                return pq16

            def prod_sel(j, zps, pq16, ytab, first, last, qeng=None):
                t12 = work.tile([128, 2, 2, KC], F16, tag="t12")
                nc.vector.tensor_tensor(
                    t12[:, 0, :, :],
                    pq16[:, 0:1, :].broadcast_to([128, 2, KC]),
                    ytab[:], OP.mult)
                nc.tensor.matmul(zps[:, 0, :], wp[:, j, :], t12[:, 0, 1, :],
                                 start=first, stop=False,
                                 skip_group_check=True)
                nc.tensor.matmul(zps[:, 1, :], wm[:, j, :], t12[:, 0, 0, :],
                                 start=first, stop=False,
                                 skip_group_check=True)
                (qeng or nc.vector).tensor_tensor(
                    t12[:, 1, :, :],
                    pq16[:, 1:2, :].broadcast_to([128, 2, KC]),
                    ytab[:], OP.mult)
                nc.tensor.matmul(zps[:, 0, :], wm[:, j, :], t12[:, 1, 0, :],
                                 start=False, stop=last,
                                 skip_group_check=True)
                nc.tensor.matmul(zps[:, 1, :], wm[:, j, :], t12[:, 1, 1, :],
                                 start=False, stop=last,
                                 skip_group_check=True)

            def jloop(zps, xtd, ytab, mid=None, gq=()):
                pqs, pq16s = {}, {}
                for j in range(MT + 2):
                    if j < MT:
                        pqs[j] = st1(j, xtd)
                    if j >= 1 and j - 1 < MT:
                        pq16s[j - 1] = cast_pq(pqs.pop(j - 1))
                    if j >= 2:
                        prod_sel(j - 2, zps, pq16s.pop(j - 2), ytab,
                                 j - 2 == 0, j - 2 == MT - 1,
                                 qeng=nc.gpsimd if (j - 2) in gq else None)
                    if mid is not None:
                        mid(j)

            # kc0 trig (shared-height chain + dup DMAs)
            with tc.high_priority():
                fr0 = trig_head(tin0, nc.vector)
                tbl0 = sin_op(fr0)
            xtd0, ytab0 = xy_tables(tbl0, dcfb0, nc.vector, nc.sync)

            # kc1 inputs + V trig head; frac on gpsimd
            tin1 = trig.tile([128, KC], F32, tag="tin")
            nc.sync.dma_start(tin1[0:64, :],
                              d_phase[1:2, KC:K].broadcast_to([64, KC]))
            nc.sync.dma_start(tin1[64:128, :],
                              d_phase[0:1, KC:K].broadcast_to([64, KC]))
            dcfb1 = trig.tile([128, 2, KC], F16, tag="dcfb")
            nc.sync.dma_start(
                dcfb1[0:64, :, :],
                d_dcf[:, KC:K].rearrange("(o a) k -> o a k", o=1).broadcast_to([64, 2, KC]))
            fr1 = trig_head(tin1, nc.gpsimd)

            nc.vector.tensor_tensor(sT[:], sxm[:, 0:ACH],
                                    sxm[:, ACH:2 * ACH], OP.mult)
            nc.vector.tensor_scalar(wm[:], wp[:], -1.0, None, OP.mult)

            zps0 = psZ.tile([128, 2, KC], F32, tag="zps")
            state = {}

            def mid0(j):
                if j == 2:
                    state["tbl1"] = sin_op(fr1)
                if j == 3:
                    state["xy1"] = xy_tables(state["tbl1"], dcfb1,
                                             nc.vector, nc.gpsimd)
            jloop(zps0, xtd0, ytab0, mid=mid0)
            xtd1, ytab1 = state["xy1"]
            zout0 = zop.tile([128, 2, KC], F16, tag="zout")
            nc.scalar.copy(zout0[:, 0, :], zps0[:, 0, :])
            nc.vector.tensor_copy(zout0[:, 1, :], zps0[:, 1, :])
            nc.gpsimd.dma_start(d_z[:, :, 0:KC], zout0[:])

            zps1 = psZ.tile([128, 2, KC], F32, tag="zps")
            jloop(zps1, xtd1, ytab1)
            zout1 = zop.tile([128, 2, KC], F16, tag="zout")
            nc.scalar.copy(zout1[:, 0, :], zps1[:, 0, :])
            nc.vector.tensor_copy(zout1[:, 1, :], zps1[:, 1, :])
            nc.sync.dma_start(d_z[:, 0, KC:K], zout1[:, 0, :])
            nc.gpsimd.dma_start(d_z[:, 1, KC:K], zout1[:, 1, :])

    nc.finalize()
    return nc


def _get_nc():
    if "nc" not in _CACHE:
        _CACHE["nc"] = _build_nc()
    return _CACHE["nc"]


def _stage_inputs(x, trj, phi, mps, sqrt_dcf):
    """Per-core input maps: layout/replication + tiny scale constants."""
    f32, f16 = np.float32, np.float16
    gy = (np.arange(H, dtype=np.float64) - H // 2)
    inv2pi = 1.0 / (2 * np.pi)

    sc = np.empty((128, 2), f32)
    sc[0:64, 0] = gy * inv2pi
    sc[64:128, 0] = gy * inv2pi
    sc[0:64, 1] = gy * inv2pi
    sc[64:128, 1] = gy * inv2pi    # gx == gy grid

    xt = np.ascontiguousarray(x.transpose(2, 0, 1))       # [w, a, h]
    xr = np.broadcast_to(xt[:, :, None, :], (W, A, C, H)).reshape(W, ACH)
    mt = np.ascontiguousarray(mps.transpose(2, 0, 1))     # [w, c, h]
    mr = np.broadcast_to(mt[:, None, :, :], (W, A, C, H)).reshape(W, ACH)
    sxm = np.concatenate([xr, mr], axis=1).astype(f16)

    # phi folded into h-reduction weights: Wp[j][(i,h),(t,c)] = phi[a_i,t]*(c_i==c)
    wp = np.zeros((128, MT, 128), f16)
    for j in range(MT):
        for i in range(2):
            ac = 2 * j + i
            a, c = ac // C, ac % C
            for t in range(T):
                wp[i * 64:(i + 1) * 64, j, t * C + c] = phi[a, t]
    in_maps = []
    for r in range(N_CORES):
        in_maps.append({
            "phase": np.ascontiguousarray(trj[r]).astype(f32),
            "sc": sc,
            "sxm": sxm,
            "dcf": np.broadcast_to(sqrt_dcf[r].astype(f16)[None, :],
                                   (2, K)).copy(),
            "wp": wp.reshape(128, MT * 128),
        })
    return in_maps


def kernel(x, trj, phi, mps, sqrt_dcf, subsamp_idx, _trace=False):
    from concourse.bass_utils import run_bass_kernel_spmd

    nc = _get_nc()
    in_maps = _stage_inputs(np.asarray(x), np.asarray(trj), np.asarray(phi),
                            np.asarray(mps), np.asarray(sqrt_dcf))
    res = run_bass_kernel_spmd(nc, in_maps, core_ids=list(range(N_CORES)),
                               trace=_trace)
    out = np.empty((T, C, K), dtype=np.complex64)
    idx = np.asarray(subsamp_idx).astype(np.int64)
    for t in range(T):
        r = int(idx[t])
        z = res.results[r]["z"].astype(np.float32)
        for c in range(C):
            out[t, c, :] = z[t * 4 + c, 0, :] + 1j * z[t * 4 + c, 1, :]
    if _trace:
        kernel._last_results = res
    return out


# revision 27
# speedup vs baseline: 1.0737x; 1.0737x over previous
"""Trainium2 Bass kernel for nn_SubspaceLinopFactory (subspace NUDFT forward).

Math (reference):
  s[a,c,h,w] = x[a,h,w] * mps[c,h,w]
  E[r,k,(h,w)] = exp(-i*(ty[k]*gy[h] + tx[k]*gx[w]))   (separable, per r)
  y[a,c,k] = sum_hw E * s ;  z[t,c,k] = sum_a phi[a,t]*y*dcf[k]
  out[t,c,k] = z from core r = subsamp_idx[t]

Sharding: trajectory r -> core r (R == 8 == n_cores).

Per-core pipeline (per 512-wide k-chunk):
  - one fused trig table [128,2,KC]: rows 0-63 y-phases, 64-127 x-phases.
    ScalarE Copy(scale=g/2pi per-partition, bias=16.5/16.75 for sin/cos
    halves), VectorE i32-cast, GpSimd mixed-dtype subtract -> frac,
    ScalarE Sin(scale=-2pi) -> fp16 sin/cos in one shot.
  - xtd = x-trig * dcf (VectorE), cos half DMA'd to partitions 0-63.
  - stage 1: row-tiled concurrent matmuls contract w=64: P (rows 0-63)
    and Q (rows 64-127) into one 2-bank PSUM tile; single ScalarE cast
    to fp16.
  - pair-products (broadcast AP): T1 = P x {sy,cy} = {D,A},
    T2 = Q x {sy,cy} = {B,C}   (VectorE, one j-slice on GpSimd).
  - h-reduction matmuls with phi pre-folded into +/-W [128,128] weights
    accumulate z[(t,c),k] re/im directly in PSUM; ScalarE cast to fp16,
    DMA out.
"""
import numpy as np

A, T, C, R, D, K, H, W = 3, 32, 4, 8, 2, 1024, 64, 64
N_CORES = 8
ACH = A * C * H          # 768
MT = ACH // 128          # 6 m-tiles
KC = 512
NKC = K // KC            # 2

_CACHE = {}


def _build_nc():
    import concourse.bacc as bacc
    import concourse.tile as tile
    import concourse.mybir as mybir

    AF = mybir.ActivationFunctionType
    OP = mybir.AluOpType
    F32 = mybir.dt.float32
    F16 = mybir.dt.float16
    I32 = mybir.dt.int32
    TWO_PI = float(2 * np.pi)

    nc = bacc.Bacc(None, target_bir_lowering=False)

    d_phase = nc.dram_tensor("phase", [2, K], F32, kind="ExternalInput")
    d_sc = nc.dram_tensor("sc", [128, 2], F32, kind="ExternalInput")
    d_sxm = nc.dram_tensor("sxm", [64, 2 * ACH], F16, kind="ExternalInput")
    d_dcf = nc.dram_tensor("dcf", [2, K], F16, kind="ExternalInput")
    d_wp = nc.dram_tensor("wp", [128, MT * 128], F16, kind="ExternalInput")
    d_z = nc.dram_tensor("z", [128, 2, K], F16, kind="ExternalOutput")

    with tile.TileContext(nc) as tc:
        with (
            tc.tile_pool(name="cst", bufs=1) as cst,
            tc.tile_pool(name="trig", bufs=2) as trig,
            tc.tile_pool(name="tbl", bufs=2) as tblp,
            tc.tile_pool(name="work", bufs=3) as work,
            tc.tile_pool(name="zo", bufs=2) as zop,
            tc.tile_pool(name="psPQ", bufs=3, space="PSUM") as psPQ,
            tc.tile_pool(name="psZ", bufs=1, space="PSUM") as psZ,
        ):
            sc = cst.tile([128, 2], F32)
            sxm = cst.tile([64, 2 * ACH], F16)
            wp = cst.tile([128, MT, 128], F16)
            wm = cst.tile([128, MT, 128], F16)
            sT = cst.tile([128, ACH], F16)

            # critical-path DMAs first on sync; bulk on gpsimd queue
            tin0 = trig.tile([128, KC], F32, tag="tin")
            nc.sync.dma_start(tin0[0:64, :],
                              d_phase[0:1, 0:KC].broadcast_to([64, KC]))
            nc.sync.dma_start(tin0[64:128, :],
                              d_phase[1:2, 0:KC].broadcast_to([64, KC]))
            dcfb0 = trig.tile([128, 2, KC], F16, tag="dcfb")
            nc.sync.dma_start(
                dcfb0[64:128, :, :],
                d_dcf[:, 0:KC].rearrange("(o a) k -> o a k", o=1).broadcast_to([64, 2, KC]))
            nc.gpsimd.dma_start(sc[:], d_sc[:])
            nc.gpsimd.dma_start(sxm[:], d_sxm[:])
            nc.gpsimd.dma_start(wp[:], d_wp[:])

            def trig_head(tin, feng):
                m = trig.tile([128, 2, KC], F32, tag="m")
                nc.vector.tensor_scalar(m[:, 0, :], tin[:], sc[:, 0:1], 16.5,
                                        OP.mult, OP.add)
                nc.vector.tensor_scalar(m[:, 1, :], tin[:], sc[:, 0:1], 16.75,
                                        OP.mult, OP.add)
                mi = trig.tile([128, 2, KC], I32, tag="mi")
                nc.vector.tensor_copy(mi[:], m[:])
                fr = trig.tile([128, 2, KC], F32, tag="fr")
                feng.tensor_tensor(fr[:], m[:], mi[:], OP.subtract)
                return fr

            def sin_op(fr):
                tbl = tblp.tile([128, 2, KC], F16, tag="tbl")
                nc.scalar.activation(tbl[:], fr[:], AF.Sin, scale=-TWO_PI)
                return tbl

            def xy_tables(tbl, dcfb, xeng, deng):
                xtd = tblp.tile([128, 2, KC], F16, tag="xtd")
                xeng.tensor_tensor(xtd[64:128, :, :], tbl[64:128, :, :],
                                   dcfb[64:128, :, :], OP.mult)
                deng.dma_start(xtd[0:64, 1, :], xtd[64:128, 1, :])
                ytab = tblp.tile([128, 2, KC], F16, tag="ytab")
                deng.dma_start(ytab[0:64, :, :], tbl[0:64, :, :])
                deng.dma_start(ytab[64:128, :, :], tbl[0:64, :, :])
                return xtd, ytab

            def st1(j, xtd):
                js = slice(j * 128, (j + 1) * 128)
                pq = psPQ.tile([128, 2, KC], F32, tag="pq")
                nc.tensor.matmul(pq[:, 0, :], sT[0:64, js],
                                 xtd[0:64, 1, :], start=True, stop=True)
                nc.tensor.matmul(pq[:, 1, :], sT[64:128, js],
                                 xtd[64:128, 0, :], start=True, stop=True)
                return pq

            def cast_pq(pq):
                pq16 = work.tile([128, 2, KC], F16, tag="pq16")
                nc.scalar.copy(pq16[:, 0, :], pq[:, 0, :])
                nc.scalar.copy(pq16[:, 1, :], pq[:, 1, :])
                return pq16

            def prod_sel(j, zps, pq16, ytab, first, last, qeng=None):
                t12 = work.tile([128, 2, 2, KC], F16, tag="t12")
                nc.vector.tensor_tensor(
                    t12[:, 0, :, :],
                    pq16[:, 0:1, :].broadcast_to([128, 2, KC]),
                    ytab[:], OP.mult)
                nc.tensor.matmul(zps[:, 0, :], wp[:, j, :], t12[:, 0, 1, :],
                                 start=first, stop=False,
                                 skip_group_check=True)
                nc.tensor.matmul(zps[:, 1, :], wm[:, j, :], t12[:, 0, 0, :],
                                 start=first, stop=False,
                                 skip_group_check=True)
                (qeng or nc.vector).tensor_tensor(
                    t12[:, 1, :, :],
                    pq16[:, 1:2, :].broadcast_to([128, 2, KC]),
                    ytab[:], OP.mult)
                nc.tensor.matmul(zps[:, 0, :], wm[:, j, :], t12[:, 1, 0, :],
                                 start=False, stop=last,
                                 skip_group_check=True)
                nc.tensor.matmul(zps[:, 1, :], wm[:, j, :], t12[:, 1, 1, :],
                                 start=False, stop=last,
                                 skip_group_check=True)

            def jloop(zps, xtd, ytab, mid=None, gq=()):
                pqs, pq16s = {}, {}
                for j in range(MT + 2):
                    if j < MT:
                        pqs[j] = st1(j, xtd)
                    if j >= 1 and j - 1 < MT:
                        pq16s[j - 1] = cast_pq(pqs.pop(j - 1))
                    if j >= 2:
                        prod_sel(j - 2, zps, pq16s.pop(j - 2), ytab,
                                 j - 2 == 0, j - 2 == MT - 1,
                                 qeng=nc.gpsimd if (j - 2) in gq else None)
                    if mid is not None:
                        mid(j)

            # kc0 trig (shared-height chain + dup DMAs)
            with tc.high_priority():
                fr0 = trig_head(tin0, nc.vector)
                tbl0 = sin_op(fr0)
            xtd0, ytab0 = xy_tables(tbl0, dcfb0, nc.vector, nc.sync)

            # kc1 inputs + V trig head; frac on gpsimd
            tin1 = trig.tile([128, KC], F32, tag="tin")
            nc.sync.dma_start(tin1[0:64, :],
                              d_phase[0:1, KC:K].broadcast_to([64, KC]))
            nc.sync.dma_start(tin1[64:128, :],
                              d_phase[1:2, KC:K].broadcast_to([64, KC]))
            dcfb1 = trig.tile([128, 2, KC], F16, tag="dcfb")
            nc.sync.dma_start(
                dcfb1[64:128, :, :],
                d_dcf[:, KC:K].rearrange("(o a) k -> o a k", o=1).broadcast_to([64, 2, KC]))
            fr1 = trig_head(tin1, nc.gpsimd)

            nc.vector.tensor_tensor(sT[0:64, :], sxm[:, 0:ACH],
                                    sxm[:, ACH:2 * ACH], OP.mult)
            nc.vector.tensor_scalar(wm[:], wp[:], -1.0, None, OP.mult)
            nc.gpsimd.dma_start(sT[64:128, :], sT[0:64, :])

            zps0 = psZ.tile([128, 2, KC], F32, tag="zps")
            state = {}

            def mid0(j):
                if j == 2:
                    state["tbl1"] = sin_op(fr1)
                if j == 3:
                    state["xy1"] = xy_tables(state["tbl1"], dcfb1,
                                             nc.vector, nc.gpsimd)
            jloop(zps0, xtd0, ytab0, mid=mid0)
            xtd1, ytab1 = state["xy1"]
            zout0 = zop.tile([128, 2, KC], F16, tag="zout")
            nc.scalar.copy(zout0[:, 0, :], zps0[:, 0, :])
            nc.vector.tensor_copy(zout0[:, 1, :], zps0[:, 1, :])
            nc.gpsimd.dma_start(d_z[:, :, 0:KC], zout0[:])

            zps1 = psZ.tile([128, 2, KC], F32, tag="zps")
            jloop(zps1, xtd1, ytab1)
            zout1 = zop.tile([128, 2, KC], F16, tag="zout")
            nc.scalar.copy(zout1[:, 0, :], zps1[:, 0, :])
            nc.vector.tensor_copy(zout1[:, 1, :], zps1[:, 1, :])
            nc.sync.dma_start(d_z[:, 0, KC:K], zout1[:, 0, :])
            nc.gpsimd.dma_start(d_z[:, 1, KC:K], zout1[:, 1, :])

    nc.finalize()
    return nc


def _get_nc():
    if "nc" not in _CACHE:
        _CACHE["nc"] = _build_nc()
    return _CACHE["nc"]


def _stage_inputs(x, trj, phi, mps, sqrt_dcf):
    """Per-core input maps: layout/replication + tiny scale constants."""
    f32, f16 = np.float32, np.float16
    gy = (np.arange(H, dtype=np.float64) - H // 2)
    inv2pi = 1.0 / (2 * np.pi)

    sc = np.empty((128, 2), f32)
    sc[0:64, 0] = gy * inv2pi
    sc[64:128, 0] = gy * inv2pi
    sc[0:64, 1] = gy * inv2pi
    sc[64:128, 1] = gy * inv2pi    # gx == gy grid

    xt = np.ascontiguousarray(x.transpose(2, 0, 1))       # [w, a, h]
    xr = np.broadcast_to(xt[:, :, None, :], (W, A, C, H)).reshape(W, ACH)
    mt = np.ascontiguousarray(mps.transpose(2, 0, 1))     # [w, c, h]
    mr = np.broadcast_to(mt[:, None, :, :], (W, A, C, H)).reshape(W, ACH)
    sxm = np.concatenate([xr, mr], axis=1).astype(f16)

    # phi folded into h-reduction weights: Wp[j][(i,h),(t,c)] = phi[a_i,t]*(c_i==c)
    wp = np.zeros((128, MT, 128), f16)
    for j in range(MT):
        for i in range(2):
            ac = 2 * j + i
            a, c = ac // C, ac % C
            for t in range(T):
                wp[i * 64:(i + 1) * 64, j, t * C + c] = phi[a, t]
    in_maps = []
    for r in range(N_CORES):
        in_maps.append({
            "phase": np.ascontiguousarray(trj[r]).astype(f32),
            "sc": sc,
            "sxm": sxm,
            "dcf": np.broadcast_to(sqrt_dcf[r].astype(f16)[None, :],
                                   (2, K)).copy(),
            "wp": wp.reshape(128, MT * 128),
        })
    return in_maps


def kernel(x, trj, phi, mps, sqrt_dcf, subsamp_idx, _trace=False):
    from concourse.bass_utils import run_bass_kernel_spmd

    nc = _get_nc()
    in_maps = _stage_inputs(np.asarray(x), np.asarray(trj), np.asarray(phi),
                            np.asarray(mps), np.asarray(sqrt_dcf))
    res = run_bass_kernel_spmd(nc, in_maps, core_ids=list(range(N_CORES)),
                               trace=_trace)
    out = np.empty((T, C, K), dtype=np.complex64)
    idx = np.asarray(subsamp_idx).astype(np.int64)
    for t in range(T):
        r = int(idx[t])
        z = res.results[r]["z"].astype(np.float32)
        for c in range(C):
            out[t, c, :] = z[t * 4 + c, 0, :] + 1j * z[t * 4 + c, 1, :]
    if _trace:
        kernel._last_results = res
    return out


# revision 28
# speedup vs baseline: 1.0851x; 1.0107x over previous
"""Trainium2 Bass kernel for nn_SubspaceLinopFactory (subspace NUDFT forward).

Math (reference):
  s[a,c,h,w] = x[a,h,w] * mps[c,h,w]
  E[r,k,(h,w)] = exp(-i*(ty[k]*gy[h] + tx[k]*gx[w]))   (separable, per r)
  y[a,c,k] = sum_hw E * s ;  z[t,c,k] = sum_a phi[a,t]*y*dcf[k]
  out[t,c,k] = z from core r = subsamp_idx[t]

Sharding: trajectory r -> core r (R == 8 == n_cores).

Per-core pipeline (per 512-wide k-chunk):
  - one fused trig table [128,2,KC]: rows 0-63 y-phases, 64-127 x-phases.
    ScalarE Copy(scale=g/2pi per-partition, bias=16.5/16.75 for sin/cos
    halves), VectorE i32-cast, GpSimd mixed-dtype subtract -> frac,
    ScalarE Sin(scale=-2pi) -> fp16 sin/cos in one shot.
  - xtd = x-trig * dcf (VectorE), cos half DMA'd to partitions 0-63.
  - stage 1: row-tiled concurrent matmuls contract w=64: P (rows 0-63)
    and Q (rows 64-127) into one 2-bank PSUM tile; single ScalarE cast
    to fp16.
  - pair-products (broadcast AP): T1 = P x {sy,cy} = {D,A},
    T2 = Q x {sy,cy} = {B,C}   (VectorE, one j-slice on GpSimd).
  - h-reduction matmuls with phi pre-folded into +/-W [128,128] weights
    accumulate z[(t,c),k] re/im directly in PSUM; ScalarE cast to fp16,
    DMA out.
"""
import numpy as np

A, T, C, R, D, K, H, W = 3, 32, 4, 8, 2, 1024, 64, 64
N_CORES = 8
ACH = A * C * H          # 768
MT = ACH // 128          # 6 m-tiles
KC = 512
NKC = K // KC            # 2

_CACHE = {}


def _build_nc():
    import concourse.bacc as bacc
    import concourse.tile as tile
    import concourse.mybir as mybir

    AF = mybir.ActivationFunctionType
    OP = mybir.AluOpType
    F32 = mybir.dt.float32
    F16 = mybir.dt.float16
    I32 = mybir.dt.int32
    TWO_PI = float(2 * np.pi)

    nc = bacc.Bacc(None, target_bir_lowering=False)

    d_phase = nc.dram_tensor("phase", [2, K], F32, kind="ExternalInput")
    d_sc = nc.dram_tensor("sc", [128, 2], F32, kind="ExternalInput")
    d_sxm = nc.dram_tensor("sxm", [64, 2 * ACH], F16, kind="ExternalInput")
    d_dcf = nc.dram_tensor("dcf", [2, K], F16, kind="ExternalInput")
    d_wp = nc.dram_tensor("wp", [128, MT * 128], F16, kind="ExternalInput")
    d_z = nc.dram_tensor("z", [128, 2, K], F16, kind="ExternalOutput")

    with tile.TileContext(nc) as tc:
        with (
            tc.tile_pool(name="cst", bufs=1) as cst,
            tc.tile_pool(name="trig", bufs=2) as trig,
            tc.tile_pool(name="tbl", bufs=2) as tblp,
            tc.tile_pool(name="work", bufs=3) as work,
            tc.tile_pool(name="zo", bufs=2) as zop,
            tc.tile_pool(name="psPQ", bufs=3, space="PSUM") as psPQ,
            tc.tile_pool(name="psZ", bufs=1, space="PSUM") as psZ,
        ):
            sc = cst.tile([128, 2], F32)
            sxm = cst.tile([64, 2 * ACH], F16)
            wp = cst.tile([128, MT, 128], F16)
            wm = cst.tile([128, MT, 128], F16)
            sT = cst.tile([128, ACH], F16)

            # critical-path DMAs first on sync; bulk on gpsimd queue
            tin0 = trig.tile([128, KC], F32, tag="tin")
            nc.sync.dma_start(tin0[0:64, :],
                              d_phase[0:1, 0:KC].broadcast_to([64, KC]))
            nc.sync.dma_start(tin0[64:128, :],
                              d_phase[1:2, 0:KC].broadcast_to([64, KC]))
            dcfb0 = trig.tile([128, 2, KC], F16, tag="dcfb")
            nc.sync.dma_start(
                dcfb0[0:64, :, :],
                d_dcf[:, 0:KC].rearrange("(o a) k -> o a k", o=1).broadcast_to([64, 2, KC]))
            nc.gpsimd.dma_start(sc[:], d_sc[:])
            nc.gpsimd.dma_start(sxm[:], d_sxm[:])
            nc.gpsimd.dma_start(wp[:], d_wp[:])

            def trig_head(tin, feng):
                m = trig.tile([128, 2, KC], F32, tag="m")
                nc.vector.tensor_scalar(m[:, 0, :], tin[:], sc[:, 0:1], 16.5,
                                        OP.mult, OP.add)
                nc.vector.tensor_scalar(m[:, 1, :], tin[:], sc[:, 0:1], 16.75,
                                        OP.mult, OP.add)
                mi = trig.tile([128, 2, KC], I32, tag="mi")
                nc.vector.tensor_copy(mi[:], m[:])
                fr = trig.tile([128, 2, KC], F32, tag="fr")
                feng.tensor_tensor(fr[:], m[:], mi[:], OP.subtract)
                return fr

            def sin_op(fr):
                tbl = tblp.tile([128, 2, KC], F16, tag="tbl")
                nc.scalar.activation(tbl[:], fr[:], AF.Sin, scale=-TWO_PI)
                return tbl

            def xy_tables(tbl, dcfb, xeng, deng):
                # x tables used raw (dcf folded into ytab); cos half to rows 0-63
                xc = tblp.tile([64, KC], F16, tag="xc")
                deng.dma_start(xc[:], tbl[64:128, 1, :])
                ytd = tblp.tile([64, 2, KC], F16, tag="ytd")
                xeng.tensor_tensor(ytd[:], tbl[0:64, :, :],
                                   dcfb[0:64, :, :], OP.mult)
                ytab = tblp.tile([128, 2, KC], F16, tag="ytab")
                deng.dma_start(ytab[0:64, :, :], ytd[:])
                deng.dma_start(ytab[64:128, :, :], ytd[:])
                return (xc, tbl), ytab

            def st1(j, xtd):
                xc, tbl = xtd
                js = slice(j * 128, (j + 1) * 128)
                pq = psPQ.tile([128, 2, KC], F32, tag="pq")
                nc.tensor.matmul(pq[:, 0, :], sT[0:64, js],
                                 xc[:], start=True, stop=True)
                nc.tensor.matmul(pq[:, 1, :], sT[64:128, js],
                                 tbl[64:128, 0, :], start=True, stop=True)
                return pq

            def cast_pq(pq):
                pq16 = work.tile([128, 2, KC], F16, tag="pq16")
                nc.scalar.copy(pq16[:, 0, :], pq[:, 0, :])
                nc.scalar.copy(pq16[:, 1, :], pq[:, 1, :])
                return pq16

            def prod_sel(j, zps, pq16, ytab, first, last, qeng=None):
                t12 = work.tile([128, 2, 2, KC], F16, tag="t12")
                nc.vector.tensor_tensor(
                    t12[:, 0, :, :],
                    pq16[:, 0:1, :].broadcast_to([128, 2, KC]),
                    ytab[:], OP.mult)
                nc.tensor.matmul(zps[:, 0, :], wp[:, j, :], t12[:, 0, 1, :],
                                 start=first, stop=False,
                                 skip_group_check=True)
                nc.tensor.matmul(zps[:, 1, :], wm[:, j, :], t12[:, 0, 0, :],
                                 start=first, stop=False,
                                 skip_group_check=True)
                (qeng or nc.vector).tensor_tensor(
                    t12[:, 1, :, :],
                    pq16[:, 1:2, :].broadcast_to([128, 2, KC]),
                    ytab[:], OP.mult)
                nc.tensor.matmul(zps[:, 0, :], wm[:, j, :], t12[:, 1, 0, :],
                                 start=False, stop=last,
                                 skip_group_check=True)
                nc.tensor.matmul(zps[:, 1, :], wm[:, j, :], t12[:, 1, 1, :],
                                 start=False, stop=last,
                                 skip_group_check=True)

            def jloop(zps, xtd, ytab, mid=None, gq=()):
                pqs, pq16s = {}, {}
                for j in range(MT + 2):
                    if j < MT:
                        pqs[j] = st1(j, xtd)
                    if j >= 1 and j - 1 < MT:
                        pq16s[j - 1] = cast_pq(pqs.pop(j - 1))
                    if j >= 2:
                        prod_sel(j - 2, zps, pq16s.pop(j - 2), ytab,
                                 j - 2 == 0, j - 2 == MT - 1,
                                 qeng=nc.gpsimd if (j - 2) in gq else None)
                    if mid is not None:
                        mid(j)

            # kc0 trig (shared-height chain + dup DMAs)
            with tc.high_priority():
                fr0 = trig_head(tin0, nc.vector)
                tbl0 = sin_op(fr0)
            xtd0, ytab0 = xy_tables(tbl0, dcfb0, nc.vector, nc.sync)

            # kc1 inputs + V trig head; frac on gpsimd
            tin1 = trig.tile([128, KC], F32, tag="tin")
            nc.sync.dma_start(tin1[0:64, :],
                              d_phase[0:1, KC:K].broadcast_to([64, KC]))
            nc.sync.dma_start(tin1[64:128, :],
                              d_phase[1:2, KC:K].broadcast_to([64, KC]))
            dcfb1 = trig.tile([128, 2, KC], F16, tag="dcfb")
            nc.sync.dma_start(
                dcfb1[0:64, :, :],
                d_dcf[:, KC:K].rearrange("(o a) k -> o a k", o=1).broadcast_to([64, 2, KC]))
            fr1 = trig_head(tin1, nc.gpsimd)

            nc.vector.tensor_tensor(sT[0:64, :], sxm[:, 0:ACH],
                                    sxm[:, ACH:2 * ACH], OP.mult)
            nc.vector.tensor_scalar(wm[:], wp[:], -1.0, None, OP.mult)
            nc.gpsimd.dma_start(sT[64:128, :], sT[0:64, :])

            zps0 = psZ.tile([128, 2, KC], F32, tag="zps")
            state = {}

            def mid0(j):
                if j == 2:
                    state["tbl1"] = sin_op(fr1)
                if j == 3:
                    state["xy1"] = xy_tables(state["tbl1"], dcfb1,
                                             nc.vector, nc.gpsimd)
            jloop(zps0, xtd0, ytab0, mid=mid0)
            xtd1, ytab1 = state["xy1"]
            zout0 = zop.tile([128, 2, KC], F16, tag="zout")
            nc.scalar.copy(zout0[:, 0, :], zps0[:, 0, :])
            nc.vector.tensor_copy(zout0[:, 1, :], zps0[:, 1, :])
            nc.gpsimd.dma_start(d_z[:, :, 0:KC], zout0[:])

            zps1 = psZ.tile([128, 2, KC], F32, tag="zps")
            jloop(zps1, xtd1, ytab1)
            zout1 = zop.tile([128, 2, KC], F16, tag="zout")
            nc.scalar.copy(zout1[:, 0, :], zps1[:, 0, :])
            nc.vector.tensor_copy(zout1[:, 1, :], zps1[:, 1, :])
            nc.sync.dma_start(d_z[:, 0, KC:K], zout1[:, 0, :])
            nc.gpsimd.dma_start(d_z[:, 1, KC:K], zout1[:, 1, :])

    nc.finalize()
    return nc


def _get_nc():
    if "nc" not in _CACHE:
        _CACHE["nc"] = _build_nc()
    return _CACHE["nc"]


def _stage_inputs(x, trj, phi, mps, sqrt_dcf):
    """Per-core input maps: layout/replication + tiny scale constants."""
    f32, f16 = np.float32, np.float16
    gy = (np.arange(H, dtype=np.float64) - H // 2)
    inv2pi = 1.0 / (2 * np.pi)

    sc = np.empty((128, 2), f32)
    sc[0:64, 0] = gy * inv2pi
    sc[64:128, 0] = gy * inv2pi
    sc[0:64, 1] = gy * inv2pi
    sc[64:128, 1] = gy * inv2pi    # gx == gy grid

    xt = np.ascontiguousarray(x.transpose(2, 0, 1))       # [w, a, h]
    xr = np.broadcast_to(xt[:, :, None, :], (W, A, C, H)).reshape(W, ACH)
    mt = np.ascontiguousarray(mps.transpose(2, 0, 1))     # [w, c, h]
    mr = np.broadcast_to(mt[:, None, :, :], (W, A, C, H)).reshape(W, ACH)
    sxm = np.concatenate([xr, mr], axis=1).astype(f16)

    # phi folded into h-reduction weights: Wp[j][(i,h),(t,c)] = phi[a_i,t]*(c_i==c)
    wp = np.zeros((128, MT, 128), f16)
    for j in range(MT):
        for i in range(2):
            ac = 2 * j + i
            a, c = ac // C, ac % C
            for t in range(T):
                wp[i * 64:(i + 1) * 64, j, t * C + c] = phi[a, t]
    in_maps = []
    for r in range(N_CORES):
        in_maps.append({
            "phase": np.ascontiguousarray(trj[r]).astype(f32),
            "sc": sc,
            "sxm": sxm,
            "dcf": np.broadcast_to(sqrt_dcf[r].astype(f16)[None, :],
                                   (2, K)).copy(),
            "wp": wp.reshape(128, MT * 128),
        })
    return in_maps


def kernel(x, trj, phi, mps, sqrt_dcf, subsamp_idx, _trace=False):
    from concourse.bass_utils import run_bass_kernel_spmd

    nc = _get_nc()
    in_maps = _stage_inputs(np.asarray(x), np.asarray(trj), np.asarray(phi),
                            np.asarray(mps), np.asarray(sqrt_dcf))
    res = run_bass_kernel_spmd(nc, in_maps, core_ids=list(range(N_CORES)),
                               trace=_trace)
    out = np.empty((T, C, K), dtype=np.complex64)
    idx = np.asarray(subsamp_idx).astype(np.int64)
    for t in range(T):
        r = int(idx[t])
        z = res.results[r]["z"].astype(np.float32)
        for c in range(C):
            out[t, c, :] = z[t * 4 + c, 0, :] + 1j * z[t * 4 + c, 1, :]
    if _trace:
        kernel._last_results = res
    return out
